# revision 1
# baseline (speedup 1.0000x reference)
"""GemmaAttention (B=2, S=2048, D=2048, H=8, KV=1, HD=256) on 8 trn2 NeuronCores.

Sharding: DP=2 over batch x TP=4 over head-pairs. Core c handles batch c//4 and
heads {2*(c%4), 2*(c%4)+1}. Each core computes its partial o_proj output
(row-parallel Wo); the host sums the 4 partials per batch (the all-reduce is
folded into the host-side unshard).

Dataflow on each core (everything float32r on the PE at full rate):
  QT[dq,s]  = Wq_sl.T @ hT   (hT = hidden[b].T, host-transposed)
  KT[dk,s]  = Wk.T   @ hT
  V[s,dv]   = (hT chunks as lhsT) @ Wv
  RoPE applied to QT/KT in the psum->SBUF drain (DVE), with 1/sqrt(HD) folded
  into the exp's scale argument.
  scoresT[k,q] = KT_chunk.T @ QT  (per head)
  expT = ACT Exp(scoresT * 1/16) (+ causal staircase / external mask)
  outT[dv,q] += V_chunk.T @ expT ; denominators via DVE accumulation of expT
  plus a ones-vector matmul partition-reduce; normalize outT by 1/sum.
  out_partial[s,:] = outTn_chunk.T @ Wo_sl   -> DMA to DRAM.
"""

import numpy as np

import concourse.bass as bass
import concourse.tile as tile
import concourse.mybir as mybir
from concourse import bacc
from concourse.bass_utils import run_bass_kernel_spmd
from concourse._compat import with_exitstack  # noqa: F401

P = 128
B, S, D = 2, 2048, 2048
H, KV, HD = 8, 1, 256
ROPE_BASE = 10000.0
NEG_BIG = -1.0e30

HEADS_PER_CORE = 2
DQ = HEADS_PER_CORE * HD          # 512 q-dims per core
DCH = D // P                      # 16 contraction chunks
SBLK = 512                        # s-tile for projection rhs / q-tile
NSBLK = S // SBLK                 # 4
NKC = S // P                      # 16 key chunks
NQCH = DQ // P                    # 4 QT partition chunks
NKCH = HD // P                    # 2 KT partition chunks

F32 = mybir.dt.float32
F32R = mybir.dt.float32r
EXP = mybir.ActivationFunctionType.Exp

# exec time of the last traced run (set by run_spmd when tracing)
LAST_EXEC_TIME_NS = None

_BUILD_CACHE = {}


def _build(causal: bool):
    nc = bacc.Bacc()

    hT = nc.declare_dram_parameter("hT", [D, S], F32R, isOutput=False)
    wq = nc.declare_dram_parameter("wq", [D, DQ], F32R, isOutput=False)
    wk = nc.declare_dram_parameter("wk", [D, HD], F32R, isOutput=False)
    wv = nc.declare_dram_parameter("wv", [D, HD], F32R, isOutput=False)
    wo = nc.declare_dram_parameter("wo", [DQ, D], F32R, isOutput=False)
    cosT = nc.declare_dram_parameter("cosT", [HD, S], F32, isOutput=False)
    sinT = nc.declare_dram_parameter("sinT", [HD, S], F32, isOutput=False)
    ones = nc.declare_dram_parameter("ones", [P, P], F32R, isOutput=False)
    ident = nc.declare_dram_parameter("ident", [P, P], F32R, isOutput=False)
    if causal:
        stair = nc.declare_dram_parameter("stair", [P, 2 * SBLK], F32, isOutput=False)
    else:
        maskT = nc.declare_dram_parameter("emaskT", [S, S], F32, isOutput=False)
    outp = nc.declare_dram_parameter("out_partial", [S, D], F32, isOutput=True)

    from contextlib import ExitStack
    from collections import deque
    with tile.TileContext(nc) as tc, ExitStack() as ctx:
        # persistent pools
        pq = ctx.enter_context(tc.tile_pool(name="pq", bufs=1))
        QT = pq.tile([P, NQCH, S], F32R, name="QT")
        KT = pq.tile([P, NKCH, S], F32R, name="KT")
        VN = pq.tile([P, NKC, HD], F32R, name="VN")
        ONES = pq.tile([P, P], F32R, name="ONES")
        IDENT = pq.tile([P, P], F32R, name="IDENT")
        ONEC = ONES[:, 0:1]
        ONER = ONES[0:1, :]

        # ---- phase A+B: projections + RoPE ----------------------------
        with tc.tile_pool(name="pw", bufs=1) as pw, \
             tc.tile_pool(name="pht", bufs=16) as pht, \
             tc.tile_pool(name="pcs", bufs=2) as pcs, \
             tc.tile_pool(name="pvt", bufs=2) as pvt, \
             tc.tile_pool(name="ptmp", bufs=2) as ptmp, \
             tc.tile_pool(name="pjp", bufs=8, space="PSUM") as pp:
            WQ = pw.tile([P, DCH, DQ], F32R, name="WQ")
            WK = pw.tile([P, DCH, HD], F32R, name="WK")
            WV = pw.tile([P, DCH, HD], F32R, name="WV")

            for sb in range(NSBLK):
                ssl = slice(sb * SBLK, (sb + 1) * SBLK)
                psq = [pp.tile([P, SBLK], F32, name="pp") for _ in range(NQCH)]
                psk = [pp.tile([P, SBLK], F32, name="pp") for _ in range(NKCH)]
                psvt = [pp.tile([P, SBLK], F32, name="pp") for _ in range(2)]
                COSb = pcs.tile([P, NKCH, SBLK], F32, name="cosb")
                SINb = pcs.tile([P, NKCH, SBLK], F32, name="sinb")
                hts = []
                for c in range(DCH):
                    ht = pht.tile([P, SBLK], F32R, name="ht")
                    hts.append(ht)
                    nc.sync.dma_start(out=ht, in_=hT[c * P:(c + 1) * P, ssl])
                    if sb == 0:
                        # weight chunks stream just behind their first use
                        nc.sync.dma_start(out=WQ[:, c, :], in_=wq[c * P:(c + 1) * P, :])
                        nc.sync.dma_start(out=WK[:, c, :], in_=wk[c * P:(c + 1) * P, :])
                        nc.sync.dma_start(out=WV[:, c, :], in_=wv[c * P:(c + 1) * P, :])
                        if c == 0:
                            nc.sync.dma_start(out=ONES, in_=ones[:, :])
                            nc.sync.dma_start(out=IDENT, in_=ident[:, :])
                    if 2 <= c < 2 + NKCH:
                        nc.sync.dma_start(out=COSb[:, c - 2, :],
                                          in_=cosT[(c - 2) * P:(c - 1) * P, ssl])
                        nc.sync.dma_start(out=SINb[:, c - 2, :],
                                          in_=sinT[(c - 2) * P:(c - 1) * P, ssl])
                    for i in range(NQCH):
                        nc.tensor.matmul(psq[i], lhsT=WQ[:, c, i * P:(i + 1) * P],
                                         rhs=ht, start=(c == 0), stop=(c == DCH - 1))
                    for j in range(NKCH):
                        nc.tensor.matmul(psk[j], lhsT=WK[:, c, j * P:(j + 1) * P],
                                         rhs=ht, start=(c == 0), stop=(c == DCH - 1))
                    for j in range(2):
                        nc.tensor.matmul(psvt[j], lhsT=WV[:, c, j * P:(j + 1) * P],
                                         rhs=ht, start=(c == 0), stop=(c == DCH - 1))
                # RoPE drains (fused psum->SBUF)
                def rope_pair(p0, p1, out0, out1):
                    c0 = COSb[:, 0, :]; c1 = COSb[:, 1, :]
                    s0 = SINb[:, 0, :]; s1 = SINb[:, 1, :]
                    t1 = ptmp.tile([P, SBLK], F32, name="t")
                    t2 = ptmp.tile([P, SBLK], F32, name="t")
                    nc.vector.tensor_mul(t1, p0, c0)
                    nc.vector.tensor_mul(t2, p1, s0)
                    nc.vector.tensor_sub(out0, t1, t2)
                    t3 = ptmp.tile([P, SBLK], F32, name="t")
                    t4 = ptmp.tile([P, SBLK], F32, name="t")
                    nc.vector.tensor_mul(t3, p1, c1)
                    nc.vector.tensor_mul(t4, p0, s1)
                    nc.vector.tensor_add(out1, t3, t4)
                for h in range(HEADS_PER_CORE):
                    rope_pair(psq[2 * h], psq[2 * h + 1],
                              QT[:, 2 * h, ssl], QT[:, 2 * h + 1, ssl])
                rope_pair(psk[0], psk[1], KT[:, 0, ssl], KT[:, 1, ssl])
                # VT drain then PE-transpose into natural V layout
                vts = []
                for j in range(2):
                    vt = pvt.tile([P, SBLK], F32R, name="vt")
                    nc.vector.tensor_copy(vt, psvt[j])
                    vts.append(vt)
                for si in range(SBLK // P):
                    for j in range(2):
                        pst = pp.tile([P, P], F32R, name="pp")
                        nc.tensor.transpose(pst, vts[j][:, si * P:(si + 1) * P], IDENT)
                        nc.vector.tensor_copy(
                            VN[:, sb * (SBLK // P) + si, j * P:(j + 1) * P], pst)

        # ---- late persistent: o_proj weights + normalized outT --------
        patt = ctx.enter_context(tc.tile_pool(name="patt", bufs=1))
        WO = patt.tile([P, NQCH, D], F32R, name="WO")
        for c in range(NQCH):
            nc.sync.dma_start(out=WO[:, c, :], in_=wo[c * P:(c + 1) * P, :])
        OUTN = patt.tile([P, NQCH, S], F32R, name="OUTN")

        # ---- phase C+D: attention + interleaved o_proj ----------------
        with tc.tile_pool(name="pexp", bufs=6) as pexp, \
             tc.tile_pool(name="pacc", bufs=4) as pacc, \
             tc.tile_pool(name="pou", bufs=8) as pou, \
             tc.tile_pool(name="pmisc", bufs=2) as pmisc, \
             tc.tile_pool(name="pmask", bufs=4) as pmask, \
             tc.tile_pool(name="pfin", bufs=3) as pfin, \
             tc.tile_pool(name="ps_s", bufs=3, space="PSUM") as ps_s, \
             tc.tile_pool(name="ps_o", bufs=2, space="PSUM") as ps_o, \
             tc.tile_pool(name="ps_r", bufs=2, space="PSUM") as ps_r, \
             tc.tile_pool(name="ps_f", bufs=1, space="PSUM") as ps_f:
            if causal:
                STAIR = pq.tile([P, 2 * SBLK], F32, name="STAIR")
                nc.sync.dma_start(out=STAIR, in_=stair[:, :])

            def emit_norm(pend):
                ou, acc, h, qb = pend
                qsl = slice(qb * SBLK, (qb + 1) * SBLK)
                pssum = ps_r.tile([P, SBLK], F32, name="pr")
                nc.tensor.matmul(pssum[0:1, :], lhsT=ONEC, rhs=acc)
                rsb = pmisc.tile([1, SBLK], F32R, name="rsb")
                with nc.allow_low_precision("f32r output is f32-width"):
                    nc.vector.reciprocal(rsb, pssum[0:1, :])
                psb = ps_r.tile([P, SBLK], F32, name="pr")
                nc.tensor.matmul(psb, lhsT=ONER, rhs=rsb)
                rbc = pmisc.tile([P, SBLK], F32R, name="rbc")
                nc.scalar.copy(rbc, psb)
                for dvc in range(2):
                    nc.vector.tensor_mul(OUTN[:, 2 * h + dvc, qsl], ou[dvc], rbc)

            def emit_oproj_quarter(qb):
                for st in range(4 * qb, 4 * qb + 4):
                    stsl = slice(st * P, (st + 1) * P)
                    for nb in range(NSBLK):
                        psf = ps_f.tile([P, SBLK], F32, name="pf")
                        for dvc in range(NQCH):
                            nc.tensor.matmul(psf, lhsT=OUTN[:, dvc, stsl],
                                             rhs=WO[:, dvc, nb * SBLK:(nb + 1) * SBLK],
                                             start=(dvc == 0), stop=(dvc == NQCH - 1))
                        fsb = pfin.tile([P, SBLK], F32, name="fsb")
                        nc.scalar.copy(fsb, psf)
                        nc.sync.dma_start(out=outp[stsl, nb * SBLK:(nb + 1) * SBLK],
                                          in_=fsb)

            pending = deque()
            for h in range(HEADS_PER_CORE):
                for qb in range(NSBLK):
                    qsl = slice(qb * SBLK, (qb + 1) * SBLK)
                    klim = 4 * (qb + 1) if causal else NKC
                    pso = [ps_o.tile([P, SBLK], F32, name="po") for _ in range(2)]
                    acc = pacc.tile([P, SBLK], F32R, name="acc")
                    for kc0 in range(0, klim, 2):
                        kcs = [kc0, kc0 + 1]
                        exs = []
                        for kc in kcs:
                            pss = ps_s.tile([P, SBLK], F32, name="ps")
                            for c in range(NKCH):
                                nc.tensor.matmul(pss,
                                                 lhsT=KT[:, c, kc * P:(kc + 1) * P],
                                                 rhs=QT[:, 2 * h + c, qsl],
                                                 start=(c == 0), stop=(c == NKCH - 1))
                            ex = pexp.tile([P, SBLK], F32R, name="ex")
                            nc.scalar.activation(ex, pss, EXP, scale=1.0 / 16.0)
                            if causal and kc >= 4 * qb:
                                delta = 128 * kc - 512 * qb
                                nc.vector.tensor_mul(ex, ex,
                                                     STAIR[:, 512 - delta:1024 - delta])
                            if not causal:
                                mt = pmask.tile([P, SBLK], F32, name="mt")
                                nc.sync.dma_start(
                                    out=mt, in_=maskT[kc * P:(kc + 1) * P, qsl])
                                nc.vector.tensor_mul(ex, ex, mt)
                            exs.append(ex)
                        for kc, ex in zip(kcs, exs):
                            if kc == 0:
                                nc.vector.tensor_copy(acc, ex)
                            else:
                                nc.vector.tensor_add(acc, acc, ex)
                        for kc, ex in zip(kcs, exs):
                            for dvc in range(2):
                                nc.tensor.matmul(pso[dvc],
                                                 lhsT=VN[:, kc, dvc * P:(dvc + 1) * P],
                                                 rhs=ex, start=(kc == 0),
                                                 stop=(kc == klim - 1))
                    ou = [pou.tile([P, SBLK], F32R, name="ou") for _ in range(2)]
                    for dvc in range(2):
                        nc.vector.tensor_copy(ou[dvc], pso[dvc])
                    pending.append((ou, acc, h, qb))
                    if len(pending) > 2:
                        p = pending.popleft()
                        emit_norm(p)
                        if p[2] == 1:
                            emit_oproj_quarter(p[3])
            while pending:
                p = pending.popleft()
                emit_norm(p)
                if p[2] == 1:
                    emit_oproj_quarter(p[3])

    nc.finalize()
    return nc


def _get_nc(causal: bool):
    key = bool(causal)
    if key not in _BUILD_CACHE:
        _BUILD_CACHE[key] = _build(causal)
    return _BUILD_CACHE[key]


def _rope_tables(position_ids_b):
    # cosT/sinT: [HD, S] fp32, transposed layout for the [d, s] dataflow
    pos = np.asarray(position_ids_b, dtype=np.float64)
    inv = 1.0 / (ROPE_BASE ** (np.arange(0, HD, 2, dtype=np.float64) / HD))
    f = pos[:, None] * inv[None, :]            # [S, HD/2]
    emb = np.concatenate([f, f], axis=1)       # [S, HD]
    cosT = np.ascontiguousarray(np.cos(emb).T.astype(np.float32))
    sinT = np.ascontiguousarray(np.sin(emb).T.astype(np.float32))
    return cosT, sinT


def _is_causal(attention_mask):
    m = np.asarray(attention_mask)
    if m.shape != (B, 1, S, S):
        return False
    tri = np.tril(np.ones((S, S), dtype=bool))
    canon = np.where(tri, np.float32(0.0), np.float32(-1e9))
    return all(np.array_equal(m[b, 0], canon) for b in range(B))


_ONES_NP = np.ones((P, P), dtype=np.float32)
_IDENT_NP = np.eye(P, dtype=np.float32)


def _stair():
    # multiplicative staircase: stair01[p, j] = 1 if (j - 512) >= p else 0
    j = np.arange(2 * SBLK)[None, :] - SBLK
    p = np.arange(P)[:, None]
    return np.where(j >= p, np.float32(1.0), np.float32(0.0)).astype(np.float32)


def kernel(hidden_state, attention_mask, position_ids, Wq, Wk, Wv, Wo,
           _trace=False, _tmpdir=None):
    global LAST_EXEC_TIME_NS
    hidden_state = np.asarray(hidden_state, dtype=np.float32)
    Wq = np.asarray(Wq, dtype=np.float32)
    Wk = np.asarray(Wk, dtype=np.float32)
    Wv = np.asarray(Wv, dtype=np.float32)
    Wo = np.asarray(Wo, dtype=np.float32)

    causal = _is_causal(attention_mask)
    nc = _get_nc(causal)

    stair = _stair() if causal else None
    in_maps = []
    per_batch = {}
    for b in range(B):
        hTb = np.ascontiguousarray(hidden_state[b].T)          # [D, S]
        cosT, sinT = _rope_tables(position_ids[b])
        mb = None
        if not causal:
            mb = np.ascontiguousarray(
                np.exp(np.asarray(attention_mask, dtype=np.float64)[b, 0].T)
                .astype(np.float32))
        per_batch[b] = (hTb, cosT, sinT, mb)

    for core in range(8):
        b = core // 4
        hp = core % 4
        hTb, cosT, sinT, mb = per_batch[b]
        im = {
            "hT": hTb,
            "ones": _ONES_NP,
            "ident": _IDENT_NP,
            "wq": np.ascontiguousarray(Wq[:, hp * DQ:(hp + 1) * DQ]),
            "wk": Wk,
            "wv": Wv,
            "wo": np.ascontiguousarray(Wo[hp * DQ:(hp + 1) * DQ, :]),
            "cosT": cosT,
            "sinT": sinT,
        }
        if causal:
            im["stair"] = stair
        else:
            im["maskT16"] = mb
        in_maps.append(im)

    res = run_bass_kernel_spmd(nc, in_maps, core_ids=list(range(8)),
                               trace=_trace, tmpdir=_tmpdir)
    LAST_EXEC_TIME_NS = res.exec_time_ns

    out = np.empty((B, S, D), dtype=np.float32)
    for b in range(B):
        acc = res.results[4 * b]["out_partial"].astype(np.float32).copy()
        for hp in range(1, 4):
            acc += res.results[4 * b + hp]["out_partial"]
        out[b] = acc
    return out



# revision 4
# speedup vs baseline: 1.7395x; 1.7395x over previous
"""GemmaAttention (B=2, S=2048, D=2048, H=8, KV=1, HD=256) on 8 trn2 NeuronCores.

Sharding: DP=2 over batch x TP=4 over head-pairs. Core c handles batch c//4 and
heads {2*(c%4), 2*(c%4)+1}. Each core computes its partial o_proj output
(row-parallel Wo); the host sums the 4 partials per batch (the all-reduce is
folded into the host-side unshard).

v3: full fp16 datapath on the PE (fp32 PSUM accumulation), phases interleaved
so the tensor engine never drains (keeps the HAM clock gate warm at 2.4 GHz):
  proj(0), proj(1), attn(0), proj(2), attn(1), oproj(0), proj(3), attn(2),
  oproj(1), attn(3), oproj(2), oproj(3)
All DRAM operands are packed host-side into partition-major [128, chunk, free]
layouts so each tensor loads with a single DMA instruction (the v2 trace
showed 52 small weight DMAs serializing on the sync queue at ~130 GB/s and
stalling the PE 39us at startup). proj(0) runs its Q chains chunk-major,
paired with c-interleaved ht/WQ/WK/WV chunk DMAs, so the PE starts ~1us in
and trickles at DMA pace instead of waiting for the full weight set.
Per-core dataflow:
  QT[dq,s] = Wq_sl.T @ hT, KT[dk,s] = Wk.T @ hT (RoPE fused into the psum
  drain on DVE, fp16 outputs); V[s,dv] built in natural layout via
  hT-chunk-stationary matmuls (no PE transposes).
  scoresT[k,q] = KT_chunk.T @ QT per head; expT = ACT Exp(scores/16) fp16
  (+ causal staircase); denominators: DVE adds -> ONES[128x128] matmul
  broadcast -> reciprocal_approx_fast; PV accumulates in psum, normalized
  on the DVE drain into OUTN fp16; o_proj rows stream to DRAM as fp16 with
  one wide DMA per 128-row block.
"""

import numpy as np

import concourse.bass as bass
import concourse.tile as tile
import concourse.mybir as mybir
from concourse import bacc
from concourse.bass_utils import run_bass_kernel_spmd
from concourse._compat import with_exitstack  # noqa: F401

P = 128
B, S, D = 2, 2048, 2048
H, KV, HD = 8, 1, 256
ROPE_BASE = 10000.0

HEADS_PER_CORE = 2
DQ = HEADS_PER_CORE * HD          # 512 q-dims per core
DCH = D // P                      # 16 contraction chunks
SBLK = 512                        # s-tile for projection rhs / q-tile
NSBLK = S // SBLK                 # 4
NKC = S // P                      # 16 key chunks
NQCH = DQ // P                    # 4 QT partition chunks
NKCH = HD // P                    # 2 KT partition chunks

F32 = mybir.dt.float32
F16 = mybir.dt.float16
EXP = mybir.ActivationFunctionType.Exp

# exec time of the last traced run (set by run_spmd when tracing)
LAST_EXEC_TIME_NS = None

_BUILD_CACHE = {}


def _build(causal: bool):
    nc = bacc.Bacc()

    # all inputs packed host-side to partition-major [128, chunk, free]
    hTd = nc.declare_dram_parameter("hT", [P, DCH, S], F16, isOutput=False)
    wqd = nc.declare_dram_parameter("wq", [P, DCH, DQ], F16, isOutput=False)
    wkd = nc.declare_dram_parameter("wk", [P, DCH, HD], F16, isOutput=False)
    wvd = nc.declare_dram_parameter("wv", [P, DCH, HD], F16, isOutput=False)
    wod = nc.declare_dram_parameter("wo", [P, NQCH, D], F16, isOutput=False)
    cosd = nc.declare_dram_parameter("cosT", [P, NKCH, S], F32, isOutput=False)
    sind = nc.declare_dram_parameter("sinT", [P, NKCH, S], F32, isOutput=False)
    ones = nc.declare_dram_parameter("ones", [P, P], F16, isOutput=False)
    if causal:
        stair = nc.declare_dram_parameter("stair", [P, 2 * SBLK], F16, isOutput=False)
    else:
        maskT = nc.declare_dram_parameter("emaskT", [S, S], F16, isOutput=False)
    outp = nc.declare_dram_parameter("out_partial", [S, D], F16, isOutput=True)

    from contextlib import ExitStack
    with tile.TileContext(nc) as tc, ExitStack() as ctx:
        pq = ctx.enter_context(tc.tile_pool(name="pq", bufs=1))
        QT = pq.tile([P, NQCH, S], F16, name="QT")
        KT = pq.tile([P, NKCH, S], F16, name="KT")
        VN = pq.tile([P, NKC, HD], F16, name="VN")
        OUTN = pq.tile([P, NQCH, S], F16, name="OUTN")
        WQ = pq.tile([P, DCH, DQ], F16, name="WQ")
        WK = pq.tile([P, DCH, HD], F16, name="WK")
        WV = pq.tile([P, DCH, HD], F16, name="WV")
        WO = pq.tile([P, NQCH, D], F16, name="WO")
        ONES = pq.tile([P, P], F16, name="ONES")
        if causal:
            STAIR = pq.tile([P, 2 * SBLK], F16, name="STAIR")

        pht = ctx.enter_context(tc.tile_pool(name="pht", bufs=2))
        pcs = ctx.enter_context(tc.tile_pool(name="pcs", bufs=2))
        ptmp = ctx.enter_context(tc.tile_pool(name="ptmp", bufs=4))
        pex = ctx.enter_context(tc.tile_pool(name="pex", bufs=16))
        pacc = ctx.enter_context(tc.tile_pool(name="pacc", bufs=2))
        prbc = ctx.enter_context(tc.tile_pool(name="prbc", bufs=2))
        pfin = ctx.enter_context(tc.tile_pool(name="pfin", bufs=3))
        if not causal:
            pmask = ctx.enter_context(tc.tile_pool(name="pmask", bufs=4))
        psA = ctx.enter_context(tc.tile_pool(name="psA", bufs=4, space="PSUM"))
        psM = ctx.enter_context(tc.tile_pool(name="psM", bufs=2, space="PSUM"))
        psO = ctx.enter_context(tc.tile_pool(name="psO", bufs=2, space="PSUM"))

        def rope_pair(p0, p1, cosb, sinb, out0, out1):
            c0 = cosb[:, 0, :]; c1 = cosb[:, 1, :]
            s0 = sinb[:, 0, :]; s1 = sinb[:, 1, :]
            t1 = ptmp.tile([P, SBLK], F32, name="t")
            t2 = ptmp.tile([P, SBLK], F32, name="t")
            nc.vector.tensor_mul(t1, p0, c0)
            nc.vector.tensor_mul(t2, p1, s0)
            nc.vector.tensor_sub(out0, t1, t2)
            t3 = ptmp.tile([P, SBLK], F32, name="t")
            t4 = ptmp.tile([P, SBLK], F32, name="t")
            nc.vector.tensor_mul(t3, p1, c1)
            nc.vector.tensor_mul(t4, p0, s1)
            nc.vector.tensor_add(out1, t3, t4)

        def emit_kv(sb, ht, cosb, sinb):
            ssl = slice(sb * SBLK, (sb + 1) * SBLK)
            kps = []
            for j in range(NKCH):
                ps = psA.tile([P, SBLK], F32, name="pp")
                for c in range(DCH):
                    nc.tensor.matmul(ps, lhsT=WK[:, c, j * P:(j + 1) * P],
                                     rhs=ht[:, c, :], start=(c == 0),
                                     stop=(c == DCH - 1))
                kps.append(ps)
            rope_pair(kps[0], kps[1], cosb, sinb, KT[:, 0, ssl], KT[:, 1, ssl])
            # V chains in natural [s, dv] layout: two s-chunks per psum bank
            for half in range(2):
                psv = psM.tile([P, 2 * HD], F32, name="pm")
                for sub in range(2):
                    sc = 2 * half + sub
                    for c in range(DCH):
                        nc.tensor.matmul(psv[:, sub * HD:(sub + 1) * HD],
                                         lhsT=ht[:, c, sc * P:(sc + 1) * P],
                                         rhs=WV[:, c, :], start=(c == 0),
                                         stop=(c == DCH - 1))
                for sub in range(2):
                    nc.scalar.copy(VN[:, sb * 4 + 2 * half + sub, :],
                                   psv[:, sub * HD:(sub + 1) * HD])

        def emit_proj0():
            # startup block: c-interleaved DMA of everything needed early,
            # Q chains chunk-major in head pairs so the PE trickles at DMA
            # pace instead of waiting for the full weight set.
            sb = 0
            ssl = slice(0, SBLK)
            ht = pht.tile([P, DCH, SBLK], F16, name="ht")
            cosb = pcs.tile([P, NKCH, SBLK], F32, name="cosb")
            sinb = pcs.tile([P, NKCH, SBLK], F32, name="sinb")
            for c in range(DCH):
                nc.sync.dma_start(out=ht[:, c, :], in_=hTd[:, c, ssl])
                nc.sync.dma_start(out=WQ[:, c, :], in_=wqd[:, c, :])
                if c < NKCH:
                    nc.sync.dma_start(out=cosb[:, c, :], in_=cosd[:, c, ssl])
                    nc.sync.dma_start(out=sinb[:, c, :], in_=sind[:, c, ssl])
                if c == 3:
                    nc.sync.dma_start(out=WK, in_=wkd[:, :, :])
                if c == 7:
                    nc.sync.dma_start(out=WV, in_=wvd[:, :, :])
            nc.sync.dma_start(out=ONES, in_=ones[:, :])
            if causal:
                nc.sync.dma_start(out=STAIR, in_=stair[:, :])
            for g in range(2):          # head-pair groups, chunk-major
                psq = [psA.tile([P, SBLK], F32, name="pp") for _ in range(2)]
                for c in range(DCH):
                    for i in range(2):
                        nc.tensor.matmul(psq[i],
                                         lhsT=WQ[:, c, (2 * g + i) * P:
                                                 (2 * g + i + 1) * P],
                                         rhs=ht[:, c, :], start=(c == 0),
                                         stop=(c == DCH - 1))
                rope_pair(psq[0], psq[1], cosb, sinb,
                          QT[:, 2 * g, ssl], QT[:, 2 * g + 1, ssl])
            emit_kv(sb, ht, cosb, sinb)

        def emit_proj(sb):
            ssl = slice(sb * SBLK, (sb + 1) * SBLK)
            ht = pht.tile([P, DCH, SBLK], F16, name="ht")
            nc.sync.dma_start(out=ht, in_=hTd[:, :, ssl])
            cosb = pcs.tile([P, NKCH, SBLK], F32, name="cosb")
            sinb = pcs.tile([P, NKCH, SBLK], F32, name="sinb")
            nc.sync.dma_start(out=cosb, in_=cosd[:, :, ssl])
            nc.sync.dma_start(out=sinb, in_=sind[:, :, ssl])
            # Q chains (rope drains trail by one pair)
            qps = []
            for i in range(NQCH):
                ps = psA.tile([P, SBLK], F32, name="pp")
                for c in range(DCH):
                    nc.tensor.matmul(ps, lhsT=WQ[:, c, i * P:(i + 1) * P],
                                     rhs=ht[:, c, :], start=(c == 0),
                                     stop=(c == DCH - 1))
                qps.append(ps)
                if i % 2 == 1:
                    rope_pair(qps[i - 1], qps[i], cosb, sinb,
                              QT[:, i - 1, ssl], QT[:, i, ssl])
            emit_kv(sb, ht, cosb, sinb)

        def emit_attn(qb):
            qsl = slice(qb * SBLK, (qb + 1) * SBLK)
            klim = 4 * (qb + 1) if causal else NKC
            for h in range(HEADS_PER_CORE):
                exs = []
                for kc in range(klim):
                    pss = psA.tile([P, SBLK], F32, name="pp")
                    for c in range(NKCH):
                        nc.tensor.matmul(pss,
                                         lhsT=KT[:, c, kc * P:(kc + 1) * P],
                                         rhs=QT[:, 2 * h + c, qsl],
                                         start=(c == 0), stop=(c == NKCH - 1))
                    ex = pex.tile([P, SBLK], F16, name="ex")
                    nc.scalar.activation(ex, pss, EXP, scale=1.0 / 16.0)
                    if causal and kc >= 4 * qb:
                        delta = 128 * kc - 512 * qb
                        nc.vector.tensor_mul(ex, ex,
                                             STAIR[:, 512 - delta:1024 - delta])
                    if not causal:
                        mt = pmask.tile([P, SBLK], F16, name="mt")
                        nc.sync.dma_start(
                            out=mt, in_=maskT[kc * P:(kc + 1) * P, qsl])
                        nc.vector.tensor_mul(ex, ex, mt)
                    exs.append(ex)
                # denominator accumulation on DVE (fp16 2x mode)
                acc = pacc.tile([P, SBLK], F16, name="acc")
                nc.vector.tensor_copy(acc, exs[0])
                for kc in range(1, klim):
                    nc.vector.tensor_add(acc, acc, exs[kc])
                # PV accumulation
                pso = [psO.tile([P, SBLK], F32, name="po") for _ in range(2)]
                for kc in range(klim):
                    for dvc in range(2):
                        nc.tensor.matmul(pso[dvc],
                                         lhsT=VN[:, kc, dvc * P:(dvc + 1) * P],
                                         rhs=exs[kc], start=(kc == 0),
                                         stop=(kc == klim - 1))
                # broadcast column-sums via ones matmul, then fast reciprocal
                bc = psM.tile([P, SBLK], F32, name="pm")
                nc.tensor.matmul(bc, lhsT=ONES, rhs=acc)
                rbc = prbc.tile([P, SBLK], F32, name="rbc")
                nc.vector.reciprocal_approx_fast(out=rbc, in_=bc)
                for dvc in range(2):
                    nc.vector.tensor_mul(OUTN[:, 2 * h + dvc, qsl],
                                         pso[dvc], rbc)

        def emit_oproj(qb):
            for st in range(4 * qb, 4 * qb + 4):
                stsl = slice(st * P, (st + 1) * P)
                fsb = pfin.tile([P, D], F16, name="fsb")
                for nb in range(NSBLK):
                    psf = psA.tile([P, SBLK], F32, name="pp")
                    for dvc in range(NQCH):
                        nc.tensor.matmul(psf, lhsT=OUTN[:, dvc, stsl],
                                         rhs=WO[:, dvc, nb * SBLK:(nb + 1) * SBLK],
                                         start=(dvc == 0), stop=(dvc == NQCH - 1))
                    # alternate drain engine to halve the serial drain latency
                    if nb % 2 == 0:
                        nc.scalar.copy(fsb[:, nb * SBLK:(nb + 1) * SBLK], psf)
                    else:
                        nc.vector.tensor_copy(fsb[:, nb * SBLK:(nb + 1) * SBLK],
                                              psf)
                nc.sync.dma_start(out=outp[stsl, :], in_=fsb)

        emit_proj0()
        emit_proj(1)
        nc.sync.dma_start(out=WO, in_=wod[:, :, :])
        emit_attn(0)
        emit_proj(2)
        emit_attn(1)
        emit_oproj(0)
        emit_proj(3)
        emit_attn(2)
        emit_oproj(1)
        emit_attn(3)
        emit_oproj(2)
        emit_oproj(3)

    nc.finalize()
    return nc


def _get_nc(causal: bool):
    key = bool(causal)
    if key not in _BUILD_CACHE:
        _BUILD_CACHE[key] = _build(causal)
    return _BUILD_CACHE[key]


def _pack(a, nch):
    # [nch*128, F] row-major -> [128, nch, F] partition-major
    a = np.ascontiguousarray(a)
    f = a.shape[1]
    return np.ascontiguousarray(a.reshape(nch, P, f).transpose(1, 0, 2))


def _rope_tables(position_ids_b):
    # cos/sin in packed [128, NKCH, S] fp32 layout for the [d, s] dataflow
    pos = np.asarray(position_ids_b, dtype=np.float64)
    inv = 1.0 / (ROPE_BASE ** (np.arange(0, HD, 2, dtype=np.float64) / HD))
    f = pos[:, None] * inv[None, :]            # [S, HD/2]
    emb = np.concatenate([f, f], axis=1)       # [S, HD]
    cosT = np.cos(emb).T.astype(np.float32)    # [HD, S]
    sinT = np.sin(emb).T.astype(np.float32)
    return _pack(cosT, NKCH), _pack(sinT, NKCH)


def _is_causal(attention_mask):
    m = np.asarray(attention_mask)
    if m.shape != (B, 1, S, S):
        return False
    tri = np.tril(np.ones((S, S), dtype=bool))
    canon = np.where(tri, np.float32(0.0), np.float32(-1e9))
    return all(np.array_equal(m[b, 0], canon) for b in range(B))


_ONES_NP = np.ones((P, P), dtype=np.float16)


def _stair():
    # multiplicative staircase: stair01[p, j] = 1 if (j - 512) >= p else 0
    j = np.arange(2 * SBLK)[None, :] - SBLK
    p = np.arange(P)[:, None]
    return np.where(j >= p, np.float16(1.0), np.float16(0.0)).astype(np.float16)


def kernel(hidden_state, attention_mask, position_ids, Wq, Wk, Wv, Wo,
           _trace=False, _tmpdir=None):
    global LAST_EXEC_TIME_NS
    hidden_state = np.asarray(hidden_state, dtype=np.float32)

    causal = _is_causal(attention_mask)
    nc = _get_nc(causal)

    Wq16 = np.asarray(Wq, dtype=np.float16)
    Wk16 = np.asarray(Wk, dtype=np.float16)
    Wv16 = np.asarray(Wv, dtype=np.float16)
    Wo16 = np.asarray(Wo, dtype=np.float16)
    wk_p = _pack(Wk16, DCH)
    wv_p = _pack(Wv16, DCH)

    stair = _stair() if causal else None
    in_maps = []
    per_batch = {}
    for b in range(B):
        hTb = _pack(hidden_state[b].T.astype(np.float16), DCH)   # [128,16,S]
        cosP, sinP = _rope_tables(position_ids[b])
        mb = None
        if not causal:
            mb = np.ascontiguousarray(
                np.exp(np.asarray(attention_mask, dtype=np.float64)[b, 0].T)
                .astype(np.float16))
        per_batch[b] = (hTb, cosP, sinP, mb)

    for core in range(8):
        b = core // 4
        hp = core % 4
        hTb, cosP, sinP, mb = per_batch[b]
        im = {
            "hT": hTb,
            "ones": _ONES_NP,
            "wq": _pack(Wq16[:, hp * DQ:(hp + 1) * DQ], DCH),
            "wk": wk_p,
            "wv": wv_p,
            "wo": _pack(Wo16[hp * DQ:(hp + 1) * DQ, :], NQCH),
            "cosT": cosP,
            "sinT": sinP,
        }
        if causal:
            im["stair"] = stair
        else:
            im["emaskT"] = mb
        in_maps.append(im)

    res = run_bass_kernel_spmd(nc, in_maps, core_ids=list(range(8)),
                               trace=_trace, tmpdir=_tmpdir)
    LAST_EXEC_TIME_NS = res.exec_time_ns

    out = np.empty((B, S, D), dtype=np.float32)
    for b in range(B):
        acc = res.results[4 * b]["out_partial"].astype(np.float32)
        for hp in range(1, 4):
            acc = acc + res.results[4 * b + hp]["out_partial"].astype(np.float32)
        out[b] = acc
    return out


# revision 5
# speedup vs baseline: 1.7608x; 1.0122x over previous
"""GemmaAttention (B=2, S=2048, D=2048, H=8, KV=1, HD=256) on 8 trn2 NeuronCores.

Sharding: DP=2 over batch x TP=4 over head-pairs. Core c handles batch c//4 and
heads {2*(c%4), 2*(c%4)+1}. Each core computes its partial o_proj output
(row-parallel Wo); the host sums the 4 partials per batch (the all-reduce is
folded into the host-side unshard).

v3: full fp16 datapath on the PE (fp32 PSUM accumulation), phases interleaved
so the tensor engine never drains (keeps the HAM clock gate warm at 2.4 GHz):
  proj(0), proj(1), attn(0), proj(2), attn(1), oproj(0), proj(3), attn(2),
  oproj(1), attn(3), oproj(2), oproj(3)
All DRAM operands are packed host-side into partition-major [128, chunk, free]
layouts so each tensor loads with a single DMA instruction (the v2 trace
showed 52 small weight DMAs serializing on the sync queue at ~130 GB/s and
stalling the PE 39us at startup). proj(0) runs its Q chains chunk-major,
paired with c-interleaved ht/WQ/WK/WV chunk DMAs, so the PE starts ~1us in
and trickles at DMA pace instead of waiting for the full weight set.
Per-core dataflow:
  QT[dq,s] = Wq_sl.T @ hT, KT[dk,s] = Wk.T @ hT (RoPE fused into the psum
  drain on DVE, fp16 outputs); V[s,dv] built in natural layout via
  hT-chunk-stationary matmuls (no PE transposes).
  scoresT[k,q] = KT_chunk.T @ QT per head; expT = ACT Exp(scores/16) fp16
  (+ causal staircase); denominators: DVE adds -> ONES[128x128] matmul
  broadcast -> reciprocal_approx_fast; PV accumulates in psum, normalized
  on the DVE drain into OUTN fp16; o_proj rows stream to DRAM as fp16 with
  one wide DMA per 128-row block.
"""

import numpy as np

import concourse.bass as bass
import concourse.tile as tile
import concourse.mybir as mybir
from concourse import bacc
from concourse.bass_utils import run_bass_kernel_spmd
from concourse._compat import with_exitstack  # noqa: F401

P = 128
B, S, D = 2, 2048, 2048
H, KV, HD = 8, 1, 256
ROPE_BASE = 10000.0

HEADS_PER_CORE = 2
DQ = HEADS_PER_CORE * HD          # 512 q-dims per core
DCH = D // P                      # 16 contraction chunks
SBLK = 512                        # s-tile for projection rhs / q-tile
NSBLK = S // SBLK                 # 4
NKC = S // P                      # 16 key chunks
NQCH = DQ // P                    # 4 QT partition chunks
NKCH = HD // P                    # 2 KT partition chunks

F32 = mybir.dt.float32
F16 = mybir.dt.float16
EXP = mybir.ActivationFunctionType.Exp

# exec time of the last traced run (set by run_spmd when tracing)
LAST_EXEC_TIME_NS = None

_BUILD_CACHE = {}


def _build(causal: bool):
    nc = bacc.Bacc()

    # all inputs packed host-side to partition-major [128, chunk, free]
    hTd = nc.declare_dram_parameter("hT", [P, DCH, S], F16, isOutput=False)
    wqd = nc.declare_dram_parameter("wq", [P, DCH, DQ], F16, isOutput=False)
    wkd = nc.declare_dram_parameter("wk", [P, DCH, HD], F16, isOutput=False)
    wvd = nc.declare_dram_parameter("wv", [P, DCH, HD], F16, isOutput=False)
    wod = nc.declare_dram_parameter("wo", [P, NQCH, D], F16, isOutput=False)
    cosd = nc.declare_dram_parameter("cosT", [P, NKCH, S], F32, isOutput=False)
    sind = nc.declare_dram_parameter("sinT", [P, NKCH, S], F32, isOutput=False)
    ones = nc.declare_dram_parameter("ones", [P, P], F16, isOutput=False)
    if causal:
        stair = nc.declare_dram_parameter("stair", [P, 2 * SBLK], F16, isOutput=False)
    else:
        maskT = nc.declare_dram_parameter("emaskT", [S, S], F16, isOutput=False)
    outp = nc.declare_dram_parameter("out_partial", [S, D], F16, isOutput=True)

    from contextlib import ExitStack
    with tile.TileContext(nc) as tc, ExitStack() as ctx:
        pq = ctx.enter_context(tc.tile_pool(name="pq", bufs=1))
        QT = pq.tile([P, NQCH, S], F16, name="QT")
        KT = pq.tile([P, NKCH, S], F16, name="KT")
        VN = pq.tile([P, NKC, HD], F16, name="VN")
        OUTN = pq.tile([P, NQCH, S], F16, name="OUTN")
        WQ = pq.tile([P, DCH, DQ], F16, name="WQ")
        WK = pq.tile([P, DCH, HD], F16, name="WK")
        WV = pq.tile([P, DCH, HD], F16, name="WV")
        WO = pq.tile([P, NQCH, D], F16, name="WO")
        ONES = pq.tile([P, P], F16, name="ONES")
        if causal:
            STAIR = pq.tile([P, 2 * SBLK], F16, name="STAIR")

        pht = ctx.enter_context(tc.tile_pool(name="pht", bufs=2))
        pcs = ctx.enter_context(tc.tile_pool(name="pcs", bufs=2))
        ptmp = ctx.enter_context(tc.tile_pool(name="ptmp", bufs=4))
        pex = ctx.enter_context(tc.tile_pool(name="pex", bufs=16))
        pacc = ctx.enter_context(tc.tile_pool(name="pacc", bufs=2))
        prbc = ctx.enter_context(tc.tile_pool(name="prbc", bufs=2))
        pfin = ctx.enter_context(tc.tile_pool(name="pfin", bufs=3))
        if not causal:
            pmask = ctx.enter_context(tc.tile_pool(name="pmask", bufs=4))
        psA = ctx.enter_context(tc.tile_pool(name="psA", bufs=4, space="PSUM"))
        psM = ctx.enter_context(tc.tile_pool(name="psM", bufs=2, space="PSUM"))
        psO = ctx.enter_context(tc.tile_pool(name="psO", bufs=2, space="PSUM"))

        def rope_pair(p0, p1, cosb, sinb, out0, out1):
            c0 = cosb[:, 0, :]; c1 = cosb[:, 1, :]
            s0 = sinb[:, 0, :]; s1 = sinb[:, 1, :]
            t1 = ptmp.tile([P, SBLK], F32, name="t")
            t2 = ptmp.tile([P, SBLK], F32, name="t")
            nc.vector.tensor_mul(t1, p0, c0)
            nc.vector.tensor_mul(t2, p1, s0)
            nc.vector.tensor_sub(out0, t1, t2)
            t3 = ptmp.tile([P, SBLK], F32, name="t")
            t4 = ptmp.tile([P, SBLK], F32, name="t")
            nc.vector.tensor_mul(t3, p1, c1)
            nc.vector.tensor_mul(t4, p0, s1)
            nc.vector.tensor_add(out1, t3, t4)

        def emit_kv(sb, ht, cosb, sinb):
            ssl = slice(sb * SBLK, (sb + 1) * SBLK)
            kps = []
            for j in range(NKCH):
                ps = psA.tile([P, SBLK], F32, name="pp")
                for c in range(DCH):
                    nc.tensor.matmul(ps, lhsT=WK[:, c, j * P:(j + 1) * P],
                                     rhs=ht[:, c, :], start=(c == 0),
                                     stop=(c == DCH - 1))
                kps.append(ps)
            rope_pair(kps[0], kps[1], cosb, sinb, KT[:, 0, ssl], KT[:, 1, ssl])
            # V chains in natural [s, dv] layout: two s-chunks per psum bank
            for half in range(2):
                psv = psM.tile([P, 2 * HD], F32, name="pm")
                for sub in range(2):
                    sc = 2 * half + sub
                    for c in range(DCH):
                        nc.tensor.matmul(psv[:, sub * HD:(sub + 1) * HD],
                                         lhsT=ht[:, c, sc * P:(sc + 1) * P],
                                         rhs=WV[:, c, :], start=(c == 0),
                                         stop=(c == DCH - 1))
                for sub in range(2):
                    nc.scalar.copy(VN[:, sb * 4 + 2 * half + sub, :],
                                   psv[:, sub * HD:(sub + 1) * HD])

        def emit_proj0():
            # startup block: c-interleaved DMA of everything needed early,
            # Q chains chunk-major in head pairs so the PE trickles at DMA
            # pace instead of waiting for the full weight set.
            sb = 0
            ssl = slice(0, SBLK)
            ht = pht.tile([P, DCH, SBLK], F16, name="ht")
            cosb = pcs.tile([P, NKCH, SBLK], F32, name="cosb")
            sinb = pcs.tile([P, NKCH, SBLK], F32, name="sinb")
            for c in range(DCH):
                nc.sync.dma_start(out=ht[:, c, :], in_=hTd[:, c, ssl])
                nc.sync.dma_start(out=WQ[:, c, :], in_=wqd[:, c, :])
                if c in (9, 10):
                    nc.sync.dma_start(out=cosb[:, c - 9, :], in_=cosd[:, c - 9, ssl])
                    nc.sync.dma_start(out=sinb[:, c - 9, :], in_=sind[:, c - 9, ssl])
                if c == 11:
                    nc.sync.dma_start(out=WK, in_=wkd[:, :, :])
                if c == 13:
                    nc.sync.dma_start(out=WV, in_=wvd[:, :, :])
            nc.sync.dma_start(out=ONES, in_=ones[:, :])
            if causal:
                nc.sync.dma_start(out=STAIR, in_=stair[:, :])
            for g in range(2):          # head-pair groups, chunk-major
                psq = [psA.tile([P, SBLK], F32, name="pp") for _ in range(2)]
                for c in range(DCH):
                    for i in range(2):
                        nc.tensor.matmul(psq[i],
                                         lhsT=WQ[:, c, (2 * g + i) * P:
                                                 (2 * g + i + 1) * P],
                                         rhs=ht[:, c, :], start=(c == 0),
                                         stop=(c == DCH - 1))
                rope_pair(psq[0], psq[1], cosb, sinb,
                          QT[:, 2 * g, ssl], QT[:, 2 * g + 1, ssl])
            emit_kv(sb, ht, cosb, sinb)

        def emit_proj(sb):
            ssl = slice(sb * SBLK, (sb + 1) * SBLK)
            ht = pht.tile([P, DCH, SBLK], F16, name="ht")
            nc.sync.dma_start(out=ht, in_=hTd[:, :, ssl])
            cosb = pcs.tile([P, NKCH, SBLK], F32, name="cosb")
            sinb = pcs.tile([P, NKCH, SBLK], F32, name="sinb")
            nc.sync.dma_start(out=cosb, in_=cosd[:, :, ssl])
            nc.sync.dma_start(out=sinb, in_=sind[:, :, ssl])
            # Q chains (rope drains trail by one pair)
            qps = []
            for i in range(NQCH):
                ps = psA.tile([P, SBLK], F32, name="pp")
                for c in range(DCH):
                    nc.tensor.matmul(ps, lhsT=WQ[:, c, i * P:(i + 1) * P],
                                     rhs=ht[:, c, :], start=(c == 0),
                                     stop=(c == DCH - 1))
                qps.append(ps)
                if i % 2 == 1:
                    rope_pair(qps[i - 1], qps[i], cosb, sinb,
                              QT[:, i - 1, ssl], QT[:, i, ssl])
            emit_kv(sb, ht, cosb, sinb)

        def emit_attn(qb):
            qsl = slice(qb * SBLK, (qb + 1) * SBLK)
            klim = 4 * (qb + 1) if causal else NKC
            for h in range(HEADS_PER_CORE):
                # diagonal tiles only compute the unmasked q-slice [delta:]
                deltas = [max(0, 128 * kc - 512 * qb) if causal else 0
                          for kc in range(klim)]
                exs = []
                for kc in range(klim):
                    dl = deltas[kc]
                    pss = psA.tile([P, SBLK], F32, name="pp")
                    for c in range(NKCH):
                        nc.tensor.matmul(pss[:, dl:],
                                         lhsT=KT[:, c, kc * P:(kc + 1) * P],
                                         rhs=QT[:, 2 * h + c,
                                                qb * SBLK + dl:(qb + 1) * SBLK],
                                         start=(c == 0), stop=(c == NKCH - 1))
                    ex = pex.tile([P, SBLK], F16, name="ex")
                    nc.scalar.activation(ex[:, dl:], pss[:, dl:], EXP,
                                         scale=1.0 / 16.0)
                    if causal and kc >= 4 * qb:
                        # only the leading 128 cols of the slice are partial
                        nc.vector.tensor_mul(ex[:, dl:dl + P], ex[:, dl:dl + P],
                                             STAIR[:, 512:512 + P])
                    if not causal:
                        mt = pmask.tile([P, SBLK], F16, name="mt")
                        nc.sync.dma_start(
                            out=mt, in_=maskT[kc * P:(kc + 1) * P, qsl])
                        nc.vector.tensor_mul(ex, ex, mt)
                    exs.append(ex)
                # denominator accumulation on DVE (fp16 2x mode)
                acc = pacc.tile([P, SBLK], F16, name="acc")
                nc.vector.tensor_copy(acc, exs[0])
                for kc in range(1, klim):
                    dl = deltas[kc]
                    nc.vector.tensor_add(acc[:, dl:], acc[:, dl:],
                                         exs[kc][:, dl:])
                # PV accumulation
                pso = [psO.tile([P, SBLK], F32, name="po") for _ in range(2)]
                for kc in range(klim):
                    dl = deltas[kc]
                    for dvc in range(2):
                        nc.tensor.matmul(pso[dvc][:, dl:],
                                         lhsT=VN[:, kc, dvc * P:(dvc + 1) * P],
                                         rhs=exs[kc][:, dl:], start=(kc == 0),
                                         stop=(kc == klim - 1),
                                         skip_group_check=(dl > 0))
                # broadcast column-sums via ones matmul, then fast reciprocal
                bc = psM.tile([P, SBLK], F32, name="pm")
                nc.tensor.matmul(bc, lhsT=ONES, rhs=acc)
                rbc = prbc.tile([P, SBLK], F32, name="rbc")
                nc.vector.reciprocal_approx_fast(out=rbc, in_=bc)
                for dvc in range(2):
                    nc.vector.tensor_mul(OUTN[:, 2 * h + dvc, qsl],
                                         pso[dvc], rbc)

        def emit_oproj(qb, act_only=False):
            for st in range(4 * qb, 4 * qb + 4):
                stsl = slice(st * P, (st + 1) * P)
                fsb = pfin.tile([P, D], F16, name="fsb")
                for nb in range(NSBLK):
                    psf = psA.tile([P, SBLK], F32, name="pp")
                    for dvc in range(NQCH):
                        nc.tensor.matmul(psf, lhsT=OUTN[:, dvc, stsl],
                                         rhs=WO[:, dvc, nb * SBLK:(nb + 1) * SBLK],
                                         start=(dvc == 0), stop=(dvc == NQCH - 1))
                    # alternate drain engine mid-run; tail blocks stay on ACT
                    # so the DVE finishes the last attention norms sooner
                    if act_only or nb % 2 == 0:
                        nc.scalar.copy(fsb[:, nb * SBLK:(nb + 1) * SBLK], psf)
                    else:
                        nc.vector.tensor_copy(fsb[:, nb * SBLK:(nb + 1) * SBLK],
                                              psf)
                nc.sync.dma_start(out=outp[stsl, :], in_=fsb)

        emit_proj0()
        emit_proj(1)
        nc.sync.dma_start(out=WO, in_=wod[:, :, :])
        emit_attn(0)
        emit_proj(2)
        emit_attn(1)
        emit_oproj(0)
        emit_proj(3)
        emit_attn(2)
        emit_oproj(1)
        emit_attn(3)
        emit_oproj(2, act_only=True)
        emit_oproj(3, act_only=True)

    nc.finalize()
    return nc


def _get_nc(causal: bool):
    key = bool(causal)
    if key not in _BUILD_CACHE:
        _BUILD_CACHE[key] = _build(causal)
    return _BUILD_CACHE[key]


def _pack(a, nch):
    # [nch*128, F] row-major -> [128, nch, F] partition-major
    a = np.ascontiguousarray(a)
    f = a.shape[1]
    return np.ascontiguousarray(a.reshape(nch, P, f).transpose(1, 0, 2))


def _rope_tables(position_ids_b):
    # cos/sin in packed [128, NKCH, S] fp32 layout for the [d, s] dataflow
    pos = np.asarray(position_ids_b, dtype=np.float64)
    inv = 1.0 / (ROPE_BASE ** (np.arange(0, HD, 2, dtype=np.float64) / HD))
    f = pos[:, None] * inv[None, :]            # [S, HD/2]
    emb = np.concatenate([f, f], axis=1)       # [S, HD]
    cosT = np.cos(emb).T.astype(np.float32)    # [HD, S]
    sinT = np.sin(emb).T.astype(np.float32)
    return _pack(cosT, NKCH), _pack(sinT, NKCH)


def _is_causal(attention_mask):
    m = np.asarray(attention_mask)
    if m.shape != (B, 1, S, S):
        return False
    tri = np.tril(np.ones((S, S), dtype=bool))
    canon = np.where(tri, np.float32(0.0), np.float32(-1e9))
    return all(np.array_equal(m[b, 0], canon) for b in range(B))


_ONES_NP = np.ones((P, P), dtype=np.float16)


def _stair():
    # multiplicative staircase: stair01[p, j] = 1 if (j - 512) >= p else 0
    j = np.arange(2 * SBLK)[None, :] - SBLK
    p = np.arange(P)[:, None]
    return np.where(j >= p, np.float16(1.0), np.float16(0.0)).astype(np.float16)


def kernel(hidden_state, attention_mask, position_ids, Wq, Wk, Wv, Wo,
           _trace=False, _tmpdir=None):
    global LAST_EXEC_TIME_NS
    hidden_state = np.asarray(hidden_state, dtype=np.float32)

    causal = _is_causal(attention_mask)
    nc = _get_nc(causal)

    Wq16 = np.asarray(Wq, dtype=np.float16)
    Wk16 = np.asarray(Wk, dtype=np.float16)
    Wv16 = np.asarray(Wv, dtype=np.float16)
    Wo16 = np.asarray(Wo, dtype=np.float16)
    wk_p = _pack(Wk16, DCH)
    wv_p = _pack(Wv16, DCH)

    stair = _stair() if causal else None
    in_maps = []
    per_batch = {}
    for b in range(B):
        hTb = _pack(hidden_state[b].T.astype(np.float16), DCH)   # [128,16,S]
        cosP, sinP = _rope_tables(position_ids[b])
        mb = None
        if not causal:
            mb = np.ascontiguousarray(
                np.exp(np.asarray(attention_mask, dtype=np.float64)[b, 0].T)
                .astype(np.float16))
        per_batch[b] = (hTb, cosP, sinP, mb)

    for core in range(8):
        b = core // 4
        hp = core % 4
        hTb, cosP, sinP, mb = per_batch[b]
        im = {
            "hT": hTb,
            "ones": _ONES_NP,
            "wq": _pack(Wq16[:, hp * DQ:(hp + 1) * DQ], DCH),
            "wk": wk_p,
            "wv": wv_p,
            "wo": _pack(Wo16[hp * DQ:(hp + 1) * DQ, :], NQCH),
            "cosT": cosP,
            "sinT": sinP,
        }
        if causal:
            im["stair"] = stair
        else:
            im["emaskT"] = mb
        in_maps.append(im)

    res = run_bass_kernel_spmd(nc, in_maps, core_ids=list(range(8)),
                               trace=_trace, tmpdir=_tmpdir)
    LAST_EXEC_TIME_NS = res.exec_time_ns

    out = np.empty((B, S, D), dtype=np.float32)
    for b in range(B):
        acc = res.results[4 * b]["out_partial"].astype(np.float32)
        for hp in range(1, 4):
            acc = acc + res.results[4 * b + hp]["out_partial"].astype(np.float32)
        out[b] = acc
    return out


# revision 6
# speedup vs baseline: 1.7609x; 1.0000x over previous
"""GemmaAttention (B=2, S=2048, D=2048, H=8, KV=1, HD=256) on 8 trn2 NeuronCores.

Sharding: DP=2 over batch x TP=4 over head-pairs. Core c handles batch c//4 and
heads {2*(c%4), 2*(c%4)+1}. Each core computes its partial o_proj output
(row-parallel Wo); the host sums the 4 partials per batch (the all-reduce is
folded into the host-side unshard).

v3: full fp16 datapath on the PE (fp32 PSUM accumulation), phases interleaved
so the tensor engine never drains (keeps the HAM clock gate warm at 2.4 GHz):
  proj(0), proj(1), attn(0), proj(2), attn(1), oproj(0), proj(3), attn(2),
  oproj(1), attn(3), oproj(2), oproj(3)
All DRAM operands are packed host-side into partition-major [128, chunk, free]
layouts so each tensor loads with a single DMA instruction (the v2 trace
showed 52 small weight DMAs serializing on the sync queue at ~130 GB/s and
stalling the PE 39us at startup). proj(0) runs its Q chains chunk-major,
paired with c-interleaved ht/WQ/WK/WV chunk DMAs, so the PE starts ~1us in
and trickles at DMA pace instead of waiting for the full weight set.
Per-core dataflow:
  QT[dq,s] = Wq_sl.T @ hT, KT[dk,s] = Wk.T @ hT (RoPE fused into the psum
  drain on DVE, fp16 outputs); V[s,dv] built in natural layout via
  hT-chunk-stationary matmuls (no PE transposes).
  scoresT[k,q] = KT_chunk.T @ QT per head; expT = ACT Exp(scores/16) fp16
  (+ causal staircase); denominators: DVE adds -> ONES[128x128] matmul
  broadcast -> reciprocal_approx_fast; PV accumulates in psum, normalized
  on the DVE drain into OUTN fp16; o_proj rows stream to DRAM as fp16 with
  one wide DMA per 128-row block.
"""

import numpy as np

import concourse.bass as bass
import concourse.tile as tile
import concourse.mybir as mybir
from concourse import bacc
from concourse.bass_utils import run_bass_kernel_spmd
from concourse._compat import with_exitstack  # noqa: F401

P = 128
B, S, D = 2, 2048, 2048
H, KV, HD = 8, 1, 256
ROPE_BASE = 10000.0

HEADS_PER_CORE = 2
DQ = HEADS_PER_CORE * HD          # 512 q-dims per core
DCH = D // P                      # 16 contraction chunks
SBLK = 512                        # s-tile for projection rhs / q-tile
NSBLK = S // SBLK                 # 4
NKC = S // P                      # 16 key chunks
NQCH = DQ // P                    # 4 QT partition chunks
NKCH = HD // P                    # 2 KT partition chunks

F32 = mybir.dt.float32
F16 = mybir.dt.float16
EXP = mybir.ActivationFunctionType.Exp

# exec time of the last traced run (set by run_spmd when tracing)
LAST_EXEC_TIME_NS = None

_BUILD_CACHE = {}


def _build(causal: bool):
    nc = bacc.Bacc()

    # all inputs packed host-side to partition-major [128, chunk, free]
    hTd = nc.declare_dram_parameter("hT", [P, DCH, S], F16, isOutput=False)
    wqd = nc.declare_dram_parameter("wq", [P, DCH, DQ], F16, isOutput=False)
    wkd = nc.declare_dram_parameter("wk", [P, DCH, HD], F16, isOutput=False)
    wvd = nc.declare_dram_parameter("wv", [P, DCH, HD], F16, isOutput=False)
    wod = nc.declare_dram_parameter("wo", [P, NQCH, D], F16, isOutput=False)
    cosd = nc.declare_dram_parameter("cosT", [P, NKCH, S], F32, isOutput=False)
    sind = nc.declare_dram_parameter("sinT", [P, NKCH, S], F32, isOutput=False)
    ones = nc.declare_dram_parameter("ones", [P, P], F16, isOutput=False)
    if causal:
        stair = nc.declare_dram_parameter("stair", [P, 2 * SBLK], F16, isOutput=False)
    else:
        maskT = nc.declare_dram_parameter("emaskT", [S, S], F16, isOutput=False)
    outp = nc.declare_dram_parameter("out_partial", [S, D], F16, isOutput=True)

    from contextlib import ExitStack
    with tile.TileContext(nc) as tc, ExitStack() as ctx:
        pq = ctx.enter_context(tc.tile_pool(name="pq", bufs=1))
        QT = pq.tile([P, NQCH, S], F16, name="QT")
        KT = pq.tile([P, NKCH, S], F16, name="KT")
        VN = pq.tile([P, NKC, HD], F16, name="VN")
        OUTN = pq.tile([P, NQCH, S], F16, name="OUTN")
        WQ = pq.tile([P, DCH, DQ], F16, name="WQ")
        WK = pq.tile([P, DCH, HD], F16, name="WK")
        WV = pq.tile([P, DCH, HD], F16, name="WV")
        WO = pq.tile([P, NQCH, D], F16, name="WO")
        ONES = pq.tile([P, P], F16, name="ONES")
        if causal:
            STAIR = pq.tile([P, 2 * SBLK], F16, name="STAIR")

        pht = ctx.enter_context(tc.tile_pool(name="pht", bufs=2))
        pcs = ctx.enter_context(tc.tile_pool(name="pcs", bufs=2))
        ptmp = ctx.enter_context(tc.tile_pool(name="ptmp", bufs=4))
        pex = ctx.enter_context(tc.tile_pool(name="pex", bufs=16))
        pacc = ctx.enter_context(tc.tile_pool(name="pacc", bufs=2))
        prbc = ctx.enter_context(tc.tile_pool(name="prbc", bufs=2))
        pfin = ctx.enter_context(tc.tile_pool(name="pfin", bufs=3))
        if not causal:
            pmask = ctx.enter_context(tc.tile_pool(name="pmask", bufs=4))
        psA = ctx.enter_context(tc.tile_pool(name="psA", bufs=4, space="PSUM"))
        psM = ctx.enter_context(tc.tile_pool(name="psM", bufs=2, space="PSUM"))
        psO = ctx.enter_context(tc.tile_pool(name="psO", bufs=2, space="PSUM"))

        def rope_pair(p0, p1, cosb, sinb, out0, out1):
            c0 = cosb[:, 0, :]; c1 = cosb[:, 1, :]
            s0 = sinb[:, 0, :]; s1 = sinb[:, 1, :]
            t1 = ptmp.tile([P, SBLK], F32, name="t")
            t2 = ptmp.tile([P, SBLK], F32, name="t")
            nc.vector.tensor_mul(t1, p0, c0)
            nc.vector.tensor_mul(t2, p1, s0)
            nc.vector.tensor_sub(out0, t1, t2)
            t3 = ptmp.tile([P, SBLK], F32, name="t")
            t4 = ptmp.tile([P, SBLK], F32, name="t")
            nc.vector.tensor_mul(t3, p1, c1)
            nc.vector.tensor_mul(t4, p0, s1)
            nc.vector.tensor_add(out1, t3, t4)

        def emit_kv(sb, ht, cosb, sinb):
            ssl = slice(sb * SBLK, (sb + 1) * SBLK)
            kps = []
            for j in range(NKCH):
                ps = psA.tile([P, SBLK], F32, name="pp")
                for c in range(DCH):
                    nc.tensor.matmul(ps, lhsT=WK[:, c, j * P:(j + 1) * P],
                                     rhs=ht[:, c, :], start=(c == 0),
                                     stop=(c == DCH - 1))
                kps.append(ps)
            rope_pair(kps[0], kps[1], cosb, sinb, KT[:, 0, ssl], KT[:, 1, ssl])
            # V chains in natural [s, dv] layout: two s-chunks per psum bank
            for half in range(2):
                psv = psM.tile([P, 2 * HD], F32, name="pm")
                for sub in range(2):
                    sc = 2 * half + sub
                    for c in range(DCH):
                        nc.tensor.matmul(psv[:, sub * HD:(sub + 1) * HD],
                                         lhsT=ht[:, c, sc * P:(sc + 1) * P],
                                         rhs=WV[:, c, :], start=(c == 0),
                                         stop=(c == DCH - 1))
                for sub in range(2):
                    nc.scalar.copy(VN[:, sb * 4 + 2 * half + sub, :],
                                   psv[:, sub * HD:(sub + 1) * HD])

        def emit_proj0():
            # startup block: c-interleaved DMA of everything needed early,
            # Q chains chunk-major in head pairs so the PE trickles at DMA
            # pace instead of waiting for the full weight set.
            sb = 0
            ssl = slice(0, SBLK)
            ht = pht.tile([P, DCH, SBLK], F16, name="ht")
            cosb = pcs.tile([P, NKCH, SBLK], F32, name="cosb")
            sinb = pcs.tile([P, NKCH, SBLK], F32, name="sinb")
            for c in range(DCH):
                nc.sync.dma_start(out=ht[:, c, :], in_=hTd[:, c, ssl])
                nc.sync.dma_start(out=WQ[:, c, :], in_=wqd[:, c, :])
                if c in (9, 10):
                    nc.sync.dma_start(out=cosb[:, c - 9, :], in_=cosd[:, c - 9, ssl])
                    nc.sync.dma_start(out=sinb[:, c - 9, :], in_=sind[:, c - 9, ssl])
                if c == 11:
                    nc.sync.dma_start(out=WK, in_=wkd[:, :, :])
                if c == 13:
                    nc.sync.dma_start(out=WV, in_=wvd[:, :, :])
            nc.sync.dma_start(out=ONES, in_=ones[:, :])
            if causal:
                nc.sync.dma_start(out=STAIR, in_=stair[:, :])
            for g in range(2):          # head-pair groups, chunk-major
                psq = [psA.tile([P, SBLK], F32, name="pp") for _ in range(2)]
                for c in range(DCH):
                    for i in range(2):
                        nc.tensor.matmul(psq[i],
                                         lhsT=WQ[:, c, (2 * g + i) * P:
                                                 (2 * g + i + 1) * P],
                                         rhs=ht[:, c, :], start=(c == 0),
                                         stop=(c == DCH - 1))
                rope_pair(psq[0], psq[1], cosb, sinb,
                          QT[:, 2 * g, ssl], QT[:, 2 * g + 1, ssl])
            emit_kv(sb, ht, cosb, sinb)

        def emit_proj(sb):
            ssl = slice(sb * SBLK, (sb + 1) * SBLK)
            ht = pht.tile([P, DCH, SBLK], F16, name="ht")
            if sb == 1:
                # fine-grained chunks: the startup DMA stream is still
                # draining, so let the first Q chain trickle chunk-by-chunk
                for c in range(DCH):
                    nc.sync.dma_start(out=ht[:, c, :], in_=hTd[:, c, ssl])
            else:
                nc.sync.dma_start(out=ht, in_=hTd[:, :, ssl])
            cosb = pcs.tile([P, NKCH, SBLK], F32, name="cosb")
            sinb = pcs.tile([P, NKCH, SBLK], F32, name="sinb")
            nc.sync.dma_start(out=cosb, in_=cosd[:, :, ssl])
            nc.sync.dma_start(out=sinb, in_=sind[:, :, ssl])
            # Q chains (rope drains trail by one pair)
            qps = []
            for i in range(NQCH):
                ps = psA.tile([P, SBLK], F32, name="pp")
                for c in range(DCH):
                    nc.tensor.matmul(ps, lhsT=WQ[:, c, i * P:(i + 1) * P],
                                     rhs=ht[:, c, :], start=(c == 0),
                                     stop=(c == DCH - 1))
                qps.append(ps)
                if i % 2 == 1:
                    rope_pair(qps[i - 1], qps[i], cosb, sinb,
                              QT[:, i - 1, ssl], QT[:, i, ssl])
            emit_kv(sb, ht, cosb, sinb)

        def emit_attn(qb, heads=(0, 1)):
            qsl = slice(qb * SBLK, (qb + 1) * SBLK)
            klim = 4 * (qb + 1) if causal else NKC
            for h in heads:
                # diagonal tiles only compute the unmasked q-slice [delta:]
                deltas = [max(0, 128 * kc - 512 * qb) if causal else 0
                          for kc in range(klim)]
                exs = []
                for kc in range(klim):
                    dl = deltas[kc]
                    pss = psA.tile([P, SBLK], F32, name="pp")
                    for c in range(NKCH):
                        nc.tensor.matmul(pss[:, dl:],
                                         lhsT=KT[:, c, kc * P:(kc + 1) * P],
                                         rhs=QT[:, 2 * h + c,
                                                qb * SBLK + dl:(qb + 1) * SBLK],
                                         start=(c == 0), stop=(c == NKCH - 1))
                    ex = pex.tile([P, SBLK], F16, name="ex")
                    nc.scalar.activation(ex[:, dl:], pss[:, dl:], EXP,
                                         scale=1.0 / 16.0)
                    if causal and kc >= 4 * qb:
                        # only the leading 128 cols of the slice are partial
                        nc.vector.tensor_mul(ex[:, dl:dl + P], ex[:, dl:dl + P],
                                             STAIR[:, 512:512 + P])
                    if not causal:
                        mt = pmask.tile([P, SBLK], F16, name="mt")
                        nc.sync.dma_start(
                            out=mt, in_=maskT[kc * P:(kc + 1) * P, qsl])
                        nc.vector.tensor_mul(ex, ex, mt)
                    exs.append(ex)
                # denominator accumulation on DVE (fp16 2x mode)
                acc = pacc.tile([P, SBLK], F16, name="acc")
                nc.vector.tensor_copy(acc, exs[0])
                for kc in range(1, klim):
                    dl = deltas[kc]
                    nc.vector.tensor_add(acc[:, dl:], acc[:, dl:],
                                         exs[kc][:, dl:])
                # PV accumulation
                pso = [psO.tile([P, SBLK], F32, name="po") for _ in range(2)]
                for kc in range(klim):
                    dl = deltas[kc]
                    for dvc in range(2):
                        nc.tensor.matmul(pso[dvc][:, dl:],
                                         lhsT=VN[:, kc, dvc * P:(dvc + 1) * P],
                                         rhs=exs[kc][:, dl:], start=(kc == 0),
                                         stop=(kc == klim - 1),
                                         skip_group_check=(dl > 0))
                # broadcast column-sums via ones matmul, then fast reciprocal
                bc = psM.tile([P, SBLK], F32, name="pm")
                nc.tensor.matmul(bc, lhsT=ONES, rhs=acc)
                rbc = prbc.tile([P, SBLK], F32, name="rbc")
                nc.vector.reciprocal_approx_fast(out=rbc, in_=bc)
                for dvc in range(2):
                    nc.vector.tensor_mul(OUTN[:, 2 * h + dvc, qsl],
                                         pso[dvc], rbc)

        def emit_oproj(qb, act_only=False):
            for st in range(4 * qb, 4 * qb + 4):
                stsl = slice(st * P, (st + 1) * P)
                fsb = pfin.tile([P, D], F16, name="fsb")
                for nb in range(NSBLK):
                    psf = psA.tile([P, SBLK], F32, name="pp")
                    for dvc in range(NQCH):
                        nc.tensor.matmul(psf, lhsT=OUTN[:, dvc, stsl],
                                         rhs=WO[:, dvc, nb * SBLK:(nb + 1) * SBLK],
                                         start=(dvc == 0), stop=(dvc == NQCH - 1))
                    # alternate drain engine mid-run; tail blocks stay on ACT
                    # so the DVE finishes the last attention norms sooner
                    if act_only or nb % 2 == 0:
                        nc.scalar.copy(fsb[:, nb * SBLK:(nb + 1) * SBLK], psf)
                    else:
                        nc.vector.tensor_copy(fsb[:, nb * SBLK:(nb + 1) * SBLK],
                                              psf)
                nc.sync.dma_start(out=outp[stsl, :], in_=fsb)

        emit_proj0()
        emit_proj(1)
        nc.sync.dma_start(out=WO, in_=wod[:, :, :])
        emit_attn(0)
        emit_proj(2)
        emit_attn(1)
        emit_oproj(0)
        emit_proj(3)
        emit_attn(2)
        emit_oproj(1)
        emit_attn(3, heads=(0,))
        emit_oproj(2, act_only=True)
        emit_attn(3, heads=(1,))
        emit_oproj(3, act_only=True)

    nc.finalize()
    return nc


def _get_nc(causal: bool):
    key = bool(causal)
    if key not in _BUILD_CACHE:
        _BUILD_CACHE[key] = _build(causal)
    return _BUILD_CACHE[key]


def _pack(a, nch):
    # [nch*128, F] row-major -> [128, nch, F] partition-major
    a = np.ascontiguousarray(a)
    f = a.shape[1]
    return np.ascontiguousarray(a.reshape(nch, P, f).transpose(1, 0, 2))


def _rope_tables(position_ids_b):
    # cos/sin in packed [128, NKCH, S] fp32 layout for the [d, s] dataflow
    pos = np.asarray(position_ids_b, dtype=np.float64)
    inv = 1.0 / (ROPE_BASE ** (np.arange(0, HD, 2, dtype=np.float64) / HD))
    f = pos[:, None] * inv[None, :]            # [S, HD/2]
    emb = np.concatenate([f, f], axis=1)       # [S, HD]
    cosT = np.cos(emb).T.astype(np.float32)    # [HD, S]
    sinT = np.sin(emb).T.astype(np.float32)
    return _pack(cosT, NKCH), _pack(sinT, NKCH)


def _is_causal(attention_mask):
    m = np.asarray(attention_mask)
    if m.shape != (B, 1, S, S):
        return False
    tri = np.tril(np.ones((S, S), dtype=bool))
    canon = np.where(tri, np.float32(0.0), np.float32(-1e9))
    return all(np.array_equal(m[b, 0], canon) for b in range(B))


_ONES_NP = np.ones((P, P), dtype=np.float16)


def _stair():
    # multiplicative staircase: stair01[p, j] = 1 if (j - 512) >= p else 0
    j = np.arange(2 * SBLK)[None, :] - SBLK
    p = np.arange(P)[:, None]
    return np.where(j >= p, np.float16(1.0), np.float16(0.0)).astype(np.float16)


def kernel(hidden_state, attention_mask, position_ids, Wq, Wk, Wv, Wo,
           _trace=False, _tmpdir=None):
    global LAST_EXEC_TIME_NS
    hidden_state = np.asarray(hidden_state, dtype=np.float32)

    causal = _is_causal(attention_mask)
    nc = _get_nc(causal)

    Wq16 = np.asarray(Wq, dtype=np.float16)
    Wk16 = np.asarray(Wk, dtype=np.float16)
    Wv16 = np.asarray(Wv, dtype=np.float16)
    Wo16 = np.asarray(Wo, dtype=np.float16)
    wk_p = _pack(Wk16, DCH)
    wv_p = _pack(Wv16, DCH)

    stair = _stair() if causal else None
    in_maps = []
    per_batch = {}
    for b in range(B):
        hTb = _pack(hidden_state[b].T.astype(np.float16), DCH)   # [128,16,S]
        cosP, sinP = _rope_tables(position_ids[b])
        mb = None
        if not causal:
            mb = np.ascontiguousarray(
                np.exp(np.asarray(attention_mask, dtype=np.float64)[b, 0].T)
                .astype(np.float16))
        per_batch[b] = (hTb, cosP, sinP, mb)

    for core in range(8):
        b = core // 4
        hp = core % 4
        hTb, cosP, sinP, mb = per_batch[b]
        im = {
            "hT": hTb,
            "ones": _ONES_NP,
            "wq": _pack(Wq16[:, hp * DQ:(hp + 1) * DQ], DCH),
            "wk": wk_p,
            "wv": wv_p,
            "wo": _pack(Wo16[hp * DQ:(hp + 1) * DQ, :], NQCH),
            "cosT": cosP,
            "sinT": sinP,
        }
        if causal:
            im["stair"] = stair
        else:
            im["emaskT"] = mb
        in_maps.append(im)

    res = run_bass_kernel_spmd(nc, in_maps, core_ids=list(range(8)),
                               trace=_trace, tmpdir=_tmpdir)
    LAST_EXEC_TIME_NS = res.exec_time_ns

    out = np.empty((B, S, D), dtype=np.float32)
    for b in range(B):
        acc = res.results[4 * b]["out_partial"].astype(np.float32)
        for hp in range(1, 4):
            acc = acc + res.results[4 * b + hp]["out_partial"].astype(np.float32)
        out[b] = acc
    return out
